# revision 26
# baseline (speedup 1.0000x reference)
"""FlowNet-C correlation layer (MAX_DISP=20, STRIDE=2) on 8 trn2 cores.

Strategy: shard by (batch b, output-row half). Core k handles b=k//2,
output rows [24*(k%2), 24*(k%2)+24). Contraction over C=128 runs on the
TensorEngine as banded-Gram matmuls: per output row pair (h0,h0+1) and
dy-triple g, G2[w, col] += x1_row[128,96]^T @ x2p_rows[128,408] with the
h-pair pooled via PSUM accumulation (f32r, full rate at moving>=256).

The Gram is evicted PSUM->SBUF into a [w, col, dy] layout (dy
innermost) in bf16 by 2-bank-granular copies alternating DVE/Act
(psum bufs=4 keeps the PE streaming), then written to DRAM split over
the SP and Pool DMA queues. In that layout the 1681 wanted elements
(dx, dy) for one output pixel are a single contiguous 3362B run, so
diagonal extraction is one thin 2D DMA per w-parity (SP + Act
queues). A GpSimd add folds the 2x2 pool and an SP-queue DMA writes
the row out in bf16 (host upconverts during the assembly transpose).

DMA cost on trn2 is (free-dim bytes x 0.39ns) charged to the issuing
engine, so transfers are kept partition-fat/free-thin and spread over
the SP, Act and Pool queues to stay under the PE's 4.65us/row-pair.
Stages are software-pipelined with a skew (extract rp-1, add rp-2,
out rp-3) so no engine queue head-of-line blocks, and the last
row-pair's tail is dx-chunked so its write->extract->add->out chain
overlaps. Scale 1/(4*C) is folded into x1 on the host.
"""

import os

import numpy as np

import concourse.bacc as bacc
import concourse.bass as bass
import concourse.mybir as mybir
import concourse.tile as tile
from concourse.ap import AP
from concourse.bass import MemorySpace
from concourse.bass_utils import run_bass_kernel_spmd

MD = 20
K = 41
CC = K * K            # 1681
B, C, H, W = 4, 128, 96, 96
OH, OW = 48, 48
WP = W + 2 * MD       # 136
HH = 48               # full-res rows per core
NOH = 24              # output row-pairs per core
ROWS = HH + 2 * MD    # 88 x2p rows needed per core (h_local+dy <= 47+40)
GFREE = WP * K        # 5576 = per-partition Gram elems in [col, dy] layout

F32 = mybir.dt.float32
F32R = mybir.dt.float32r
BF16 = mybir.dt.bfloat16

LAST_EXEC_NS = None
_CACHED = None


def _build_nc():
    nc = bacc.Bacc("TRN2", target_bir_lowering=False)
    x1d = nc.dram_tensor("x1h", [C, HH * W], F32R, kind="ExternalInput")
    x2d = nc.dram_tensor("x2p", [C, ROWS * WP], F32R, kind="ExternalInput")
    outd = nc.dram_tensor("out", [NOH * OW, CC], BF16, kind="ExternalOutput")

    with tile.TileContext(nc) as tc:
        with (
            tc.tile_pool(name="inp", bufs=1) as inp_pool,
            tc.tile_pool(name="gsb", bufs=2) as gs_pool,
            tc.tile_pool(name="dd", bufs=2) as d_pool,
            tc.tile_pool(name="st", bufs=2) as s_pool,
            tc.tile_pool(name="ps", bufs=4, space=MemorySpace.PSUM) as psum_pool,
            tc.tile_pool(name="dr", bufs=2, space=MemorySpace.DRAM) as dram_pool,
        ):
            A = inp_pool.tile([C, HH * W], F32R)
            Bt = inp_pool.tile([C, ROWS * WP], F32R)
            # TRN2 ldweights encodes only ONE semaphore wait, so matmuls must
            # only ever depend on a single sem. Funnel input readiness through
            # the DVE counter (which later matmuls inherit transitively via
            # PSUM-eviction waits): DMA into staging tiles (SP/Act queues in
            # parallel), DVE-copy into A/Bt.
            # order loads so rp0's needs (x1 rows 0-11, x2 rows 0-43) are
            # staged first; DMAs alternate SP/Act queues
            loads = [("a", 0, 12), ("b", 0, 4), ("b", 4, 8), ("b", 12, 8),
                     ("b", 20, 8), ("b", 28, 8), ("b", 36, 8), ("a", 12, 12),
                     ("b", 44, 8), ("a", 24, 12), ("b", 52, 8), ("b", 60, 8),
                     ("a", 36, 12), ("b", 68, 8), ("b", 76, 8), ("b", 84, 4)]
            if os.environ.get("CORR_DIRECT_LOADS", "1") == "1":
                # DMA straight into A/Bt: matmuls then wait on two DMA-queue
                # sems plus the evictor sem (legalized by tile as standalone
                # PE waits).
                for i, (which, r0, n) in enumerate(loads):
                    eng = (nc.sync, nc.scalar, nc.gpsimd)[i % 3]
                    if which == "a":
                        eng.dma_start(A[:, r0 * W:(r0 + n) * W],
                                      x1d[:, r0 * W:(r0 + n) * W])
                    else:
                        eng.dma_start(Bt[:, r0 * WP:(r0 + n) * WP],
                                      x2d[:, r0 * WP:(r0 + n) * WP])
            else:
                with tc.tile_pool(name="stg", bufs=4) as stage_pool:
                    for i, (which, r0) in enumerate(loads):
                        eng = nc.sync if i % 2 == 0 else nc.scalar
                        if which == "a":
                            stg = stage_pool.tile([C, 12 * W], F32R, tag="stga")
                            eng.dma_start(stg[:], x1d[:, r0 * W:(r0 + 12) * W])
                            nc.vector.tensor_copy(A[:, r0 * W:(r0 + 12) * W], stg[:])
                        else:
                            stg = stage_pool.tile([C, 8 * WP], F32R, tag="stgb")
                            eng.dma_start(stg[:], x2d[:, r0 * WP:(r0 + 8) * WP])
                            nc.vector.tensor_copy(Bt[:, r0 * WP:(r0 + 8) * WP], stg[:])

            # Per-iteration stages are software-pipelined so no engine queue
            # ever head-of-line blocks on an unmet dependency: body rp issues
            # extraction for rp-1, pool-add for rp-2 and the output DMA for
            # rp-3 (their inputs completed in earlier iterations), then the
            # current Gram write last (it waits on this body's evictions).
            gbs, des, dos, ss = {}, {}, {}, {}

            def extract(r, dx0=0, dx1=K, engs=None):
                # diagonal extraction for row-pair r: element (u, dx, dy) of
                # parity p sits at flat (2u+p)*5576 + (2u+p+dx)*41 + dy =
                # 5617p + 11234u + dx*41 + dy; the (dx, dy) block is one
                # contiguous 1681-elem (3362B) run per u -> one thin 2D DMA
                # per parity (SP+Act queues).
                gt = gbs[r][:].tensor
                if r not in des:
                    des[r] = d_pool.tile([OW, CC], BF16, tag="de", name=f"de{r}")
                    dos[r] = d_pool.tile([OW, CC], BF16, tag="do", name=f"do{r}")
                shear = [[2 * (GFREE + K), OW], [K, dx1 - dx0], [1, K]]
                eeng, oeng = engs or (nc.sync, nc.scalar)
                eeng.dma_start(des[r][:, dx0 * K:dx1 * K],
                               AP(gt, dx0 * K, shear))
                oeng.dma_start(dos[r][:, dx0 * K:dx1 * K],
                               AP(gt, GFREE + K + dx0 * K, shear))

            def pool_add(r, dx0=0, dx1=K):   # 2x2-pool finish on GpSimd
                if r not in ss:
                    ss[r] = s_pool.tile([OW, CC], BF16, tag="s", name=f"s{r}")
                sl = slice(dx0 * K, dx1 * K)
                nc.gpsimd.tensor_add(ss[r][:, sl], des[r][:, sl], dos[r][:, sl])

            def out_dma(r, dx0=0, dx1=K, eng=None):
                (eng or nc.sync).dma_start(
                    outd[r * OW:(r + 1) * OW, dx0 * K:dx1 * K],
                    ss[r][:, dx0 * K:dx1 * K])

            for rp in range(NOH):
                h0 = 2 * rp
                a0 = A[:, h0 * W:(h0 + 1) * W]
                a1 = A[:, (h0 + 1) * W:(h0 + 2) * W]
                Gsb = gs_pool.tile([96, WP, K], BF16, tag="gsb")

                def mm(ps, j, h_ap, h_off, dy0, ncols):
                    nc.tensor.matmul(
                        ps[:, j, :ncols],
                        h_ap,
                        Bt[:, (h0 + h_off + dy0) * WP:
                              (h0 + h_off + dy0) * WP + ncols],
                        start=(h_off == 0), stop=(h_off == 1),
                    )

                # skewed stages first: all their deps are already complete
                if rp >= 1:
                    engs = (nc.sync, nc.sync) if rp == NOH - 1 else None
                    extract(rp - 1, engs=engs)
                if rp >= 2:
                    pool_add(rp - 2)
                if rp >= 3:
                    out_dma(rp - 3)

                # 7 psum tiles of 2 banks each (2 dy-triples), bufs=4 so the
                # PE runs up to 4 tiles ahead of the evictions, which
                # alternate DVE/Act per tile to split the copy load.
                for t in range(7):
                    ps = psum_pool.tile([96, 2, 512], F32, tag="ps")
                    ncols_b1 = 3 * WP if t < 6 else 2 * WP
                    for hoff, hap in ((0, a0), (1, a1)):
                        mm(ps, 0, hap, hoff, 6 * t, 3 * WP)
                        mm(ps, 1, hap, hoff, 6 * t + 3, ncols_b1)
                    ev = nc.vector if t % 2 == 0 else nc.scalar
                    evf = ev.tensor_copy if t % 2 == 0 else ev.copy
                    if t < 6:
                        evf(Gsb[:, :, 6 * t:6 * t + 6].transpose([0, 2, 1]),
                            ps[:, :, :3 * WP])
                    elif rp < NOH - 1:
                        evf(Gsb[:, :, 36:39].transpose([0, 2, 1]),
                            ps[:, 0, :3 * WP])
                        evf(Gsb[:, :, 39:41].transpose([0, 2, 1]),
                            ps[:, 1, :2 * WP])
                    else:
                        nc.vector.tensor_copy(
                            Gsb[:, :, 36:39].transpose([0, 2, 1]),
                            ps[:, 0, :3 * WP])
                        nc.scalar.copy(
                            Gsb[:, :, 39:41].transpose([0, 2, 1]),
                            ps[:, 1, :2 * WP])

                # Gram (bf16, [w, col*41+dy]) -> DRAM, split SP/Pool queues;
                # issued last: it waits on this body's evictions. The final
                # body splits 3 ways so the drain starts sooner.
                Gb = dram_pool.tile([96, GFREE], BF16, tag="gb")
                gbs[rp] = Gb
                if rp < NOH - 1:
                    nc.sync.dma_start(Gb[:, :45 * K], Gsb[:, :45, :])
                    nc.gpsimd.dma_start(Gb[:, 45 * K:], Gsb[:, 45:, :])
                else:
                    # keep SP free of the last write so the drain's
                    # extractions are not head-of-line blocked behind it
                    nc.scalar.dma_start(Gb[:, :45 * K], Gsb[:, :45, :])
                    nc.gpsimd.dma_start(Gb[:, 45 * K:90 * K], Gsb[:, 45:90, :])
                    nc.scalar.dma_start(Gb[:, 90 * K:], Gsb[:, 90:, :])

            # drain the pipeline; the last row-pair's tail stages are chunked
            # along dx so extract/add/out overlap instead of serializing.
            L = NOH - 1
            CH = [(0, 21), (21, 41)]
            out_dma(L - 2)
            pool_add(L - 1)
            for dx0, dx1 in CH:
                extract(L, dx0, dx1)
            out_dma(L - 1, eng=nc.scalar)
            for i, (dx0, dx1) in enumerate(CH):
                pool_add(L, dx0, dx1)
                out_dma(L, dx0, dx1, eng=nc.sync if i % 2 == 0 else nc.scalar)
    nc.compile()
    return nc


def kernel(x1: np.ndarray, x2: np.ndarray) -> np.ndarray:
    global LAST_EXEC_NS, _CACHED
    x1 = np.ascontiguousarray(np.asarray(x1, dtype=np.float32)) * np.float32(1.0 / (4 * C))
    x2 = np.asarray(x2, dtype=np.float32)
    x2p = np.zeros((B, C, 2 * MD + H, WP), dtype=np.float32)
    x2p[:, :, MD:MD + H, MD:MD + W] = x2

    if _CACHED is None:
        _CACHED = _build_nc()
    nc = _CACHED

    in_maps = []
    for core in range(8):
        b, half = core // 2, core % 2
        a = np.ascontiguousarray(
            x1[b, :, half * HH:(half + 1) * HH, :].reshape(C, HH * W))
        x2s = np.ascontiguousarray(
            x2p[b, :, half * HH:half * HH + ROWS, :].reshape(C, ROWS * WP))
        in_maps.append({"x1h": a, "x2p": x2s})

    try:
        res = run_bass_kernel_spmd(
            nc, in_maps, core_ids=list(range(8)),
            trace=os.environ.get("CORR_TRACE") == "1",
        )
    except ImportError:
        res = run_bass_kernel_spmd(nc, in_maps, core_ids=list(range(8)))
    LAST_EXEC_NS = res.exec_time_ns

    out = np.empty((B, CC, OH, OW), dtype=np.float32)
    for core in range(8):
        b, half = core // 2, core % 2
        r = np.asarray(res.results[core]["out"]).reshape(NOH, OW, CC)
        out[b, :, half * NOH:(half + 1) * NOH, :] = r.transpose(2, 0, 1)
    return out
